# revision 1
# baseline (speedup 1.0000x reference)
"""AttnBlock (GroupNorm + single-head 1x1-conv attention + residual) on 8
Trainium2 NeuronCores.

Sharding: data-parallel over batch (4) x sequence-parallel over query tokens
(2 halves of 4096). Each core receives its batch element with the spatial
columns rotated so that its 2048 query tokens are always columns 0:2047 —
attention is invariant to key order, so one shared NEFF serves all cores.

Compute dtype: fp16 on the PE (full-rate), fp32 PSUM accumulation, fp32
softmax denominators and GroupNorm statistics.
"""

import numpy as np

P = 128
C = 512
KC = C // P          # 4 channel chunks of 128
N = 4096             # tokens (64*64)
NH = N // 2          # query tokens per core
G = 32               # groupnorm groups
GS = C // G          # 16 channels per group
EPS = 1e-6
N_CORES = 8

_CACHE = {}


def _apply_walrus_workarounds():
    """The walrus build in this container rejects any instruction carrying
    more than one semaphore wait ("Too many sync wait commands"). Split extra
    waits onto same-engine single-wait NOPs committed just before, and split
    the final TileContext drain the same way."""
    import concourse.tile as tile
    from concourse import mybir

    if getattr(tile.TileContext, "_walrus_wait_split", False):
        return

    _orig_commit = tile.TileContext._commit_instruction

    def _split_waits_commit(self, inst, lazy_reg_writes=True):
        si = inst.sync_info
        if si is not None and si.on_wait and len(si.on_wait) > 1 \
                and inst.engine != mybir.EngineType.Unassigned:
            waits = list(si.on_wait)
            si.on_wait = waits[-1:]
            for w in waits[:-1]:
                nop = mybir.InstNoOp(
                    name=self.nc.get_next_instruction_name(),
                    engine=inst.engine,
                    sync_info=mybir.SyncInfo(on_wait=[w], on_update=[]),
                    bass_nofuse=True,
                )
                _orig_commit(self, nop, lazy_reg_writes=False)
        return _orig_commit(self, inst, lazy_reg_writes=lazy_reg_writes)

    def _split_drain_and_barrier(self, tick_clock, wait_clock):
        nc = self.nc
        drain_inst = nc.sync.drain()
        wait_clock.add_sem_waits(
            drain_inst.ins, tile.ScopedClock({None: tick_clock.global_clock})
        )
        si = drain_inst.ins.sync_info
        waits = list(si.on_wait) if si is not None else []
        if len(waits) > 1:
            si.on_wait = waits[:1]
            for w in waits[1:]:
                d2 = nc.sync.drain()
                d2.ins.sync_info = mybir.SyncInfo(on_wait=[w], on_update=[])

        import os
        nc.all_engine_barrier()
        assert self.sems is not None
        popped = nc._tile_sem_poison_stack.pop()
        assert popped is self._sem_poison
        if os.environ.get("KERNEL_SKIP_SEM_RESET") != "1":
            nc.clear_and_free_semaphores(list(self.sems.allocated().values()))
            nc.all_engine_barrier()

    tile.TileContext._commit_instruction = _split_waits_commit
    tile.TileContext._drain_and_barrier = _split_drain_and_barrier
    tile.TileContext._walrus_wait_split = True


def _build():
    """Trace the Bass/Tile program once; returns the Bass module."""
    import concourse.bass as bass
    import concourse.tile as tile
    from concourse import mybir

    _apply_walrus_workarounds()

    DT = mybir.dt.float16
    F32 = mybir.dt.float32

    nc = bass.Bass("TRN2", target_bir_lowering=False, debug=False, num_devices=1)

    xr = nc.dram_tensor("xr", [C, N], DT, kind="ExternalInput").ap()
    wq = nc.dram_tensor("wq", [C, C], DT, kind="ExternalInput").ap()
    wk = nc.dram_tensor("wk", [C, C], DT, kind="ExternalInput").ap()
    wv = nc.dram_tensor("wv", [C, C], DT, kind="ExternalInput").ap()
    wo = nc.dram_tensor("wo", [C, C], DT, kind="ExternalInput").ap()
    # packed per-channel vectors: [bq, bk, bo, gamma, beta]
    bvec = nc.dram_tensor("bvec", [5, C], F32, kind="ExternalInput").ap()
    gavg = nc.dram_tensor("gavg", [P, P], F32, kind="ExternalInput").ap()
    ident = nc.dram_tensor("ident", [P, P], DT, kind="ExternalInput").ap()
    y = nc.dram_tensor("y", [C, NH], F32, kind="ExternalOutput").ap()

    xr_t = xr.rearrange("(kc p) n -> kc p n", p=P)     # [4, 128, 4096]
    y_t = y.rearrange("(oc p) n -> oc p n", p=P)       # [4, 128, 2048]

    IB = NH // P        # 16 query blocks per core
    JQ = N // 512       # 8 key chunks of 512
    NHQ = NH // 512     # 4 query-token chunks of 512

    with tile.TileContext(nc) as tc:
        import contextlib
        ctx = contextlib.ExitStack()
        with ctx:
            consts = ctx.enter_context(tc.tile_pool(name="consts", bufs=1))
            big = ctx.enter_context(tc.tile_pool(name="big", bufs=1))
            small = ctx.enter_context(tc.tile_pool(name="small", bufs=4))
            epool = ctx.enter_context(tc.tile_pool(name="epool", bufs=3))
            rpool = ctx.enter_context(tc.tile_pool(name="rpool", bufs=3))
            ps = ctx.enter_context(tc.tile_pool(name="ps", bufs=8, space="PSUM"))

            # ---- phase 1: GroupNorm -> hn (fp16) --------------------------
            # x stays fully resident in SBUF (also serves the phase-4
            # residual). x DMAs are traced first so they win the early HBM
            # bandwidth; consts ride the gpsimd SWDGE queue instead.
            hn = big.tile([P, KC, N], DT, tag="ho")
            x_full = big.tile([P, KC, N], DT, tag="xf")
            bv_sb = None
            for kc in range(KC):
                x_c = x_full[:, kc, :]
                nc.sync.dma_start(x_c[:], xr_t[kc])
                # raw per-partition sum (DVE, 2x on fp16) and sum of squares
                # (ScalarE Square with fused accumulator; hn[:, kc] is
                # throwaway scratch, overwritten by the real hn below). The
                # 1/(GS*N) normalization is folded into the host gavg matrix.
                mv2 = small.tile([P, 2], F32, tag="mv2")
                nc.vector.tensor_reduce(
                    mv2[:, 0:1], x_c[:], mybir.AxisListType.X,
                    mybir.AluOpType.add)
                nc.scalar.activation(
                    hn[:, kc, :], x_c[:], mybir.ActivationFunctionType.Square,
                    accum_out=mv2[:, 1:2])
                if bv_sb is None:
                    bv_sb = consts.tile([P, 5, KC], F32, tag="bvec")
                    nc.gpsimd.dma_start(
                        bv_sb[:], bvec.rearrange("v (kc p) -> p v kc", p=P))
                    b_sb = {n: bv_sb[:, vi, :] for vi, n in
                            enumerate(("bq", "bk", "bo", "gam", "bet"))}
                    gavg_sb = consts.tile([P, P], F32, tag="gavg")
                    nc.gpsimd.dma_start(gavg_sb[:], gavg)
                    ident_sb = consts.tile([P, P], DT, tag="ident")
                    nc.gpsimd.dma_start(ident_sb[:], ident)
                    eps_sb = consts.tile([P, 1], F32, tag="eps")
                    nc.vector.memset(eps_sb[:], EPS)
                # group-average (and broadcast back to partitions) via PE
                g_ps = ps.tile([P, 2], F32, tag="mm", name=f"gn{kc}")
                nc.tensor.matmul(g_ps[:], gavg_sb[:], mv2[:], start=True, stop=True)

                # var_g = E2_g - mean_g^2 ; rstd = 1/sqrt(var_g + eps)
                g_sb = small.tile([P, 2], F32, tag="gsb")
                nc.vector.tensor_copy(g_sb[:], g_ps[:])
                var_t = small.tile([P, 1], F32, tag="var")
                nc.gpsimd.tensor_tensor(
                    var_t[:], g_sb[:, 0:1], g_sb[:, 0:1], mybir.AluOpType.mult)
                nc.gpsimd.tensor_tensor(
                    var_t[:], g_sb[:, 1:2], var_t[:], mybir.AluOpType.subtract)
                sq = small.tile([P, 1], F32, tag="sq")
                nc.scalar.activation(
                    sq[:], var_t[:], mybir.ActivationFunctionType.Sqrt,
                    bias=eps_sb[:], scale=1.0)
                rstd = small.tile([P, 1], F32, tag="rstd")
                nc.vector.reciprocal(rstd[:], sq[:])

                # scale = rstd * gamma ; shift = beta - mean_g * scale
                scl = small.tile([P, 1], F32, tag="scl")
                nc.gpsimd.tensor_tensor(
                    scl[:], rstd[:], b_sb["gam"][:, kc:kc + 1], mybir.AluOpType.mult)
                sh = small.tile([P, 1], F32, tag="sh")
                nc.gpsimd.tensor_tensor(
                    sh[:], g_sb[:, 0:1], scl[:], mybir.AluOpType.mult)
                nc.gpsimd.tensor_tensor(
                    sh[:], b_sb["bet"][:, kc:kc + 1], sh[:], mybir.AluOpType.subtract)

                nc.vector.tensor_scalar(
                    out=hn[:, kc, :], in0=x_c[:], scalar1=scl[:], scalar2=sh[:],
                    op0=mybir.AluOpType.mult, op1=mybir.AluOpType.add)

            # HAM warm-up: dummy matmuls gated on the second-to-last hn
            # chunk keep the PE busy through the idle tail of phase 1 so
            # phase 2 starts at 2.4 GHz instead of the cold 1.2 GHz.
            warm_ps = ps.tile([P, 512], F32, tag="mm", name="warm")
            for wi in range(8):
                nc.tensor.matmul(warm_ps[:], ident_sb[:], hn[:, 2, :512],
                                 start=(wi == 0), stop=(wi == 7))

            # weights (first needed by phase 2)
            w_sb = {}
            for name, ap in (("wk", wk), ("wq", wq), ("wv", wv), ("wo", wo)):
                t = consts.tile([P, KC, C], DT, tag=f"w_{name}")
                nc.gpsimd.dma_start(t[:], ap.rearrange("(kc p) o -> p kc o", p=P))
                w_sb[name] = t

            # ---- phase 2: projections ------------------------------------
            k_sb = big.tile([P, KC, N], DT, tag="k")
            q_sb = big.tile([P, KC, NH], DT, tag="q")
            vt_sb = big.tile([P, N // P, C], DT, tag="vt")

            for oc in range(KC):
                for nt in range(JQ):
                    pp = ps.tile([P, 512], F32, tag="mm")
                    for kc in range(KC):
                        nc.tensor.matmul(
                            pp[:], w_sb["wk"][:, kc, oc * P:(oc + 1) * P],
                            hn[:, kc, nt * 512:(nt + 1) * 512],
                            start=(kc == 0), stop=(kc == KC - 1))
                    nc.scalar.activation(
                        k_sb[:, oc, nt * 512:(nt + 1) * 512], pp[:],
                        mybir.ActivationFunctionType.Identity,
                        bias=b_sb["bk"][:, oc:oc + 1], scale=1.0)
            for oc in range(KC):
                for nt in range(NHQ):
                    pp = ps.tile([P, 512], F32, tag="mm")
                    for kc in range(KC):
                        nc.tensor.matmul(
                            pp[:], w_sb["wq"][:, kc, oc * P:(oc + 1) * P],
                            hn[:, kc, nt * 512:(nt + 1) * 512],
                            start=(kc == 0), stop=(kc == KC - 1))
                    nc.scalar.activation(
                        q_sb[:, oc, nt * 512:(nt + 1) * 512], pp[:],
                        mybir.ActivationFunctionType.Identity,
                        bias=b_sb["bq"][:, oc:oc + 1], scale=1.0)
            for jc in range(N // P):
                pp = ps.tile([P, 512], F32, tag="mm")
                for kc in range(KC):
                    nc.tensor.matmul(
                        pp[:], hn[:, kc, jc * P:(jc + 1) * P], w_sb["wv"][:, kc, :],
                        start=(kc == 0), stop=(kc == KC - 1))
                nc.vector.tensor_copy(vt_sb[:, jc, :], pp[:])

            # ---- phase 3: attention, 16 query blocks ---------------------
            # Flat software pipeline over global key chunks u = ib*JQ + jq:
            #   iter t:  S-matmuls + exp of chunk t
            #            transposes + AT copy + O^T matmuls of chunk t-1
            #            epilogue of block (t-2)//JQ when t-2 ends a block
            # so the PE never sits on the exp (ACT) latency of its own chunk.
            o_sb = big.tile([P, KC, NH], DT, tag="ho", name="o_sb")
            TOT = IB * JQ
            e_hold = {}
            ssum_hold = {}
            ot_hold = {}

            def stage_s(u):
                ib, jq = divmod(u, JQ)
                if jq == 0:
                    ssum_hold[ib] = small.tile([P, JQ], F32, tag="ssum", name=f"ssum{ib}")
                s_ps = ps.tile([P, 512], F32, tag="mm")
                for kc in range(KC):
                    nc.tensor.matmul(
                        s_ps[:], q_sb[:, kc, ib * P:(ib + 1) * P],
                        k_sb[:, kc, jq * 512:(jq + 1) * 512],
                        start=(kc == 0), stop=(kc == KC - 1))
                e_sb = epool.tile([P, 512], DT, tag="e")
                nc.scalar.activation(
                    e_sb[:], s_ps[:], mybir.ActivationFunctionType.Exp,
                    accum_out=ssum_hold[ib][:, jq:jq + 1])
                e_hold[u] = e_sb

            at_hold = {}

            def stage_t(u):
                e_sb = e_hold.pop(u)
                t_ps = ps.tile([P, 512], DT, tag="mm")
                for jj in range(4):
                    nc.tensor.transpose(
                        t_ps[:, jj * P:(jj + 1) * P],
                        e_sb[:, jj * P:(jj + 1) * P], ident_sb[:])
                at_sb = epool.tile([P, 4, P], DT, tag="at")
                nc.vector.tensor_copy(
                    at_sb[:], t_ps.rearrange("p (a b) -> p a b", b=P))
                at_hold[u] = at_sb

            def stage_ot(u):
                ib, jq = divmod(u, JQ)
                if jq == 0:
                    ot_hold[ib] = ps.tile([P, C], F32, tag="mm", name=f"ot{ib}")
                ot_ps = ot_hold[ib]
                at_sb = at_hold.pop(u)
                for jj in range(4):
                    nc.tensor.matmul(
                        ot_ps[:], at_sb[:, jj, :], vt_sb[:, jq * 4 + jj, :],
                        start=(jq == 0 and jj == 0),
                        stop=(jq == JQ - 1 and jj == 3))

            def stage_epi(ib):
                ssum = ssum_hold.pop(ib)
                ot_ps = ot_hold.pop(ib)
                ssum_r = small.tile([P, 1], F32, tag="ssum_r")
                nc.vector.tensor_reduce(
                    ssum_r[:], ssum[:], mybir.AxisListType.X, mybir.AluOpType.add)
                recip = small.tile([P, 1], F32, tag="recip")
                nc.vector.reciprocal(recip[:], ssum_r[:])

                ot_sb = epool.tile([P, C], DT, tag="ot")
                nc.scalar.activation(
                    ot_sb[:], ot_ps[:], mybir.ActivationFunctionType.Copy,
                    scale=recip[:])
                to_ps = ps.tile([P, C], DT, tag="mm")
                for cb in range(KC):
                    nc.tensor.transpose(
                        to_ps[:, cb * P:(cb + 1) * P],
                        ot_sb[:, cb * P:(cb + 1) * P], ident_sb[:])
                nc.vector.tensor_copy(
                    o_sb[:, :, ib * P:(ib + 1) * P],
                    to_ps.rearrange("p (a b) -> p a b", b=P))

            # ---- phase 4 (interleaved): output projection + residual -----
            def stage_out_oc(nt, oc):
                pp = ps.tile([P, 512], F32, tag="mm")
                for kc in range(KC):
                    nc.tensor.matmul(
                        pp[:], w_sb["wo"][:, kc, oc * P:(oc + 1) * P],
                        o_sb[:, kc, nt * 512:(nt + 1) * 512],
                        start=(kc == 0), stop=(kc == KC - 1))
                r_sb = rpool.tile([P, 512], F32, tag="r")
                nc.scalar.activation(
                    r_sb[:], pp[:], mybir.ActivationFunctionType.Identity,
                    bias=b_sb["bo"][:, oc:oc + 1], scale=1.0)
                nc.vector.tensor_tensor(
                    r_sb[:], r_sb[:], x_full[:, oc, nt * 512:(nt + 1) * 512],
                    mybir.AluOpType.add)
                nc.sync.dma_start(y_t[oc][:, nt * 512:(nt + 1) * 512], r_sb[:])

            for t in range(TOT + 8):
                if t < TOT:
                    stage_s(t)
                if 1 <= t <= TOT:
                    stage_t(t - 1)
                if 2 <= t <= TOT + 1:
                    stage_ot(t - 2)
                if t >= 4 and (t - 4) % JQ == JQ - 1 and (t - 4) // JQ < IB:
                    stage_epi((t - 4) // JQ)
                # wo-projection for token slice nt once blocks 4nt..4nt+3
                # are epilogued; one oc group per iteration so the final
                # burst isn't exposed at the kernel tail.
                for oc in range(KC):
                    tt = t - 5 - oc
                    if tt >= 0 and tt % (4 * JQ) == 4 * JQ - 1 \
                            and tt // (4 * JQ) < NHQ:
                        stage_out_oc(tt // (4 * JQ), oc)

    return nc


def _prep_in_maps(inputs):
    x = np.asarray(inputs["x"], np.float32).reshape(4, C, N)
    s = np.float32(C ** -0.5)
    wq = np.asarray(inputs["wq"], np.float32)
    wk = np.asarray(inputs["wk"], np.float32)
    wv = np.asarray(inputs["wv"], np.float32)
    wo = np.asarray(inputs["wo"], np.float32)
    bvec = np.stack([
        np.asarray(inputs["bq"], np.float32) * s,
        np.asarray(inputs["bk"], np.float32),
        wo @ np.asarray(inputs["bv"], np.float32)
        + np.asarray(inputs["bo"], np.float32),
        np.asarray(inputs["gamma"], np.float32),
        np.asarray(inputs["beta"], np.float32),
    ]).astype(np.float32)
    shared = {
        "wq": np.ascontiguousarray((wq * s).T).astype(np.float16),
        "wk": np.ascontiguousarray(wk.T).astype(np.float16),
        "wv": np.ascontiguousarray(wv.T).astype(np.float16),
        "wo": np.ascontiguousarray(wo.T).astype(np.float16),
        "bvec": bvec,
        "gavg": (np.kron(np.eye(P // GS, dtype=np.float32),
                         np.ones((GS, GS), np.float32)) / (GS * N)),
        "ident": np.eye(P, dtype=np.float16),
    }
    in_maps = []
    for core in range(N_CORES):
        b, half = divmod(core, 2)
        xb = x[b]
        if half == 1:
            xrot = np.ascontiguousarray(
                np.concatenate([xb[:, NH:], xb[:, :NH]], axis=1))
        else:
            xrot = np.ascontiguousarray(xb)
        in_maps.append({"xr": xrot.astype(np.float16), **shared})
    return in_maps


def kernel_run(inputs, trace=False, trace_cores=None):
    """Run on all 8 cores; returns (full_output, BassKernelResults)."""
    from concourse.bass_utils import run_bass_kernel_spmd

    if "nc" not in _CACHE:
        _CACHE["nc"] = _build()
    nc = _CACHE["nc"]
    in_maps = _prep_in_maps(inputs)
    res = run_bass_kernel_spmd(
        nc, in_maps, core_ids=list(range(N_CORES)), trace=trace,
        trace_cores=trace_cores)
    out = np.empty((4, C, N), np.float32)
    for core in range(N_CORES):
        b, half = divmod(core, 2)
        out[b][:, half * NH:(half + 1) * NH] = res.results[core]["y"]
    return out.reshape(4, C, 64, 64), res


def kernel(**inputs):
    out, _ = kernel_run(inputs, trace=False)
    return out



# revision 7
# speedup vs baseline: 1.4886x; 1.4886x over previous
"""AttnBlock (GroupNorm + single-head 1x1-conv attention + residual) on 8
Trainium2 NeuronCores.

Sharding: data-parallel over batch (4) x sequence-parallel over query tokens
(2 halves of 4096). Each core receives its batch element with the spatial
columns rotated so that its 2048 query tokens are always columns 0:2047 —
attention is invariant to key order, so one shared NEFF serves all cores.

All heavy matmuls run in fp8e4 with MatmulPerfMode.DoubleRow (2 contraction
chunks of 128 per instruction = 2x PE throughput). Attention uses the
transposed dataflow S^T[key, query]: exp is applied in [k, q] layout so no
PE transposes are needed, the softmax denominator comes from a ones-
stationary matmul, and the output projection is pre-fused into the v
projection on the host (wvo = wo @ wv), eliminating phase 4 entirely.

Scaling: weights are scaled x32 on the host to keep fp8 values in the
normal range; the 1/1024 (q,k) descale is folded into the exp scale and
the 1/32 (v') descale into the output epilogue.
"""

import numpy as np

P = 128
C = 512
KC = C // P          # 4 channel chunks of 128
N = 4096             # tokens (64*64)
NH = N // 2          # query tokens per core
G = 32               # groupnorm groups
GS = C // G          # 16 channels per group
EPS = 1e-6
N_CORES = 8
QC = NH // 512       # 4 query chunks of 512
KT = N // P          # 32 key chunks of 128
SCALE = float(C) ** -0.5
WSC = 32.0           # host-side weight scale
LN8 = 2.0794415416798357

_CACHE = {}


def _apply_walrus_workarounds():
    """The walrus build in this container rejects any instruction carrying
    more than one semaphore wait ("Too many sync wait commands"). Split extra
    waits onto same-engine single-wait NOPs committed just before, and split
    the final TileContext drain the same way."""
    import concourse.tile as tile
    from concourse import mybir

    if getattr(tile.TileContext, "_walrus_wait_split", False):
        return

    _orig_commit = tile.TileContext._commit_instruction

    def _split_waits_commit(self, inst, lazy_reg_writes=True):
        si = inst.sync_info
        if si is not None and si.on_wait and len(si.on_wait) > 1 \
                and inst.engine != mybir.EngineType.Unassigned:
            waits = list(si.on_wait)
            si.on_wait = waits[-1:]
            for w in waits[:-1]:
                nop = mybir.InstNoOp(
                    name=self.nc.get_next_instruction_name(),
                    engine=inst.engine,
                    sync_info=mybir.SyncInfo(on_wait=[w], on_update=[]),
                    bass_nofuse=True,
                )
                _orig_commit(self, nop, lazy_reg_writes=False)
        return _orig_commit(self, inst, lazy_reg_writes=lazy_reg_writes)

    def _split_drain_and_barrier(self, tick_clock, wait_clock):
        nc = self.nc
        drain_inst = nc.sync.drain()
        wait_clock.add_sem_waits(
            drain_inst.ins, tile.ScopedClock({None: tick_clock.global_clock})
        )
        si = drain_inst.ins.sync_info
        waits = list(si.on_wait) if si is not None else []
        if len(waits) > 1:
            si.on_wait = waits[:1]
            for w in waits[1:]:
                d2 = nc.sync.drain()
                d2.ins.sync_info = mybir.SyncInfo(on_wait=[w], on_update=[])

        import os
        nc.all_engine_barrier()
        assert self.sems is not None
        popped = nc._tile_sem_poison_stack.pop()
        assert popped is self._sem_poison
        if os.environ.get("KERNEL_SKIP_SEM_RESET") != "1":
            nc.clear_and_free_semaphores(list(self.sems.allocated().values()))
            nc.all_engine_barrier()

    tile.TileContext._commit_instruction = _split_waits_commit
    tile.TileContext._drain_and_barrier = _split_drain_and_barrier
    tile.TileContext._walrus_wait_split = True


def _build():
    """Trace the Bass/Tile program once; returns the Bass module."""
    import concourse.bass as bass
    import concourse.tile as tile
    from concourse import mybir

    _apply_walrus_workarounds()

    DR = mybir.MatmulPerfMode.DoubleRow
    DT8 = mybir.dt.float8e4
    DT = mybir.dt.float16
    F32 = mybir.dt.float32

    nc = bass.Bass("TRN2", target_bir_lowering=False, debug=False, num_devices=1)

    xr = nc.dram_tensor("xr", [C, N], DT, kind="ExternalInput").ap()
    wq8 = nc.dram_tensor("wq8", [C, C], DT8, kind="ExternalInput").ap()
    wk8 = nc.dram_tensor("wk8", [C, C], DT8, kind="ExternalInput").ap()
    wvo8 = nc.dram_tensor("wvo8", [C, C], DT8, kind="ExternalInput").ap()
    # packed per-channel vectors: [32*bq, wo@bv+bo, gamma, beta]
    bvec = nc.dram_tensor("bvec", [4, C], F32, kind="ExternalInput").ap()
    gavg = nc.dram_tensor("gavg", [P, P], F32, kind="ExternalInput").ap()
    ident = nc.dram_tensor("ident", [P, P], DT, kind="ExternalInput").ap()
    y = nc.dram_tensor("y", [C, NH], DT, kind="ExternalOutput").ap()

    xr_t = xr.rearrange("(kc p) n -> kc p n", p=P)     # [4, 128, 4096]
    y_t = y.rearrange("(oc p) n -> oc p n", p=P)       # [4, 128, 2048]

    with tile.TileContext(nc) as tc:
        import contextlib
        ctx = contextlib.ExitStack()
        with ctx:
            consts = ctx.enter_context(tc.tile_pool(name="consts", bufs=1))
            big = ctx.enter_context(tc.tile_pool(name="big", bufs=1))
            e2p = ctx.enter_context(tc.tile_pool(name="e2p", bufs=2))
            small = ctx.enter_context(tc.tile_pool(name="small", bufs=4))
            rp = ctx.enter_context(tc.tile_pool(name="rp", bufs=3))
            ps = ctx.enter_context(tc.tile_pool(name="ps", bufs=4, space="PSUM"))

            # ---- phase 1: GroupNorm -> hn (fp8) --------------------------
            hn = big.tile([P, KC, N], DT8, tag="hn")
            x_full = big.tile([P, KC, N], DT, tag="xf")
            bv_sb = None
            scls = []
            shs = []
            for kc in range(KC):
                x_c = x_full[:, kc, :]
                nc.sync.dma_start(x_c[:], xr_t[kc])
                mv2 = small.tile([P, 2], F32, tag="mv2")
                nc.vector.tensor_reduce(
                    mv2[:, 0:1], x_c[:], mybir.AxisListType.X,
                    mybir.AluOpType.add)
                # hn[:, kc] is throwaway scratch here, overwritten below
                nc.scalar.activation(
                    hn[:, kc, 0:N], x_c[:], mybir.ActivationFunctionType.Square,
                    accum_out=mv2[:, 1:2])
                if bv_sb is None:
                    bv_sb = consts.tile([P, 4, KC], F32, tag="bvec")
                    nc.gpsimd.dma_start(
                        bv_sb[:], bvec.rearrange("v (kc p) -> p v kc", p=P))
                    b_sb = {n: bv_sb[:, vi, :] for vi, n in
                            enumerate(("bq", "bc", "gam", "bet"))}
                    gavg_sb = consts.tile([P, P], F32, tag="gavg")
                    nc.gpsimd.dma_start(gavg_sb[:], gavg)
                    ident_sb = consts.tile([P, P], DT, tag="ident")
                    nc.gpsimd.dma_start(ident_sb[:], ident)
                    ones8_sb = consts.tile([P, 2, P], DT8, tag="ones8")
                    nc.vector.memset(ones8_sb[:], 1.0)
                    eps_sb = consts.tile([P, 1], F32, tag="eps")
                    nc.vector.memset(eps_sb[:], EPS)
                    ebias = consts.tile([P, 1], F32, tag="ebias")
                    nc.vector.memset(ebias[:], -LN8)
                # group-average (and broadcast back to partitions) via PE
                g_ps = ps.tile([P, 2], F32, tag="pair", name=f"gn{kc}")
                nc.tensor.matmul(g_ps[:], gavg_sb[:], mv2[:], start=True, stop=True)

                g_sb = small.tile([P, 2], F32, tag="gsb")
                nc.vector.tensor_copy(g_sb[:], g_ps[:])
                var_t = small.tile([P, 1], F32, tag="var")
                nc.gpsimd.tensor_tensor(
                    var_t[:], g_sb[:, 0:1], g_sb[:, 0:1], mybir.AluOpType.mult)
                nc.gpsimd.tensor_tensor(
                    var_t[:], g_sb[:, 1:2], var_t[:], mybir.AluOpType.subtract)
                sq = small.tile([P, 1], F32, tag="sq")
                nc.scalar.activation(
                    sq[:], var_t[:], mybir.ActivationFunctionType.Sqrt,
                    bias=eps_sb[:], scale=1.0)
                rstd = small.tile([P, 1], F32, tag="rstd")
                nc.vector.reciprocal(rstd[:], sq[:])

                scl = small.tile([P, 1], F32, tag="scl", name=f"scl{kc}")
                nc.gpsimd.tensor_tensor(
                    scl[:], rstd[:], b_sb["gam"][:, kc:kc + 1], mybir.AluOpType.mult)
                sh = small.tile([P, 1], F32, tag="sh", name=f"sh{kc}")
                nc.gpsimd.tensor_tensor(
                    sh[:], g_sb[:, 0:1], scl[:], mybir.AluOpType.mult)
                nc.gpsimd.tensor_tensor(
                    sh[:], b_sb["bet"][:, kc:kc + 1], sh[:], mybir.AluOpType.subtract)
                scls.append(scl)
                shs.append(sh)

            # normalize: split across ACT and DVE
            for kc in range(KC):
                if kc % 2 == 0:
                    nc.scalar.activation(
                        hn[:, kc, :], x_full[:, kc, :],
                        mybir.ActivationFunctionType.Identity,
                        bias=shs[kc][:], scale=scls[kc][:])
                else:
                    nc.vector.tensor_scalar(
                        out=hn[:, kc, :], in0=x_full[:, kc, :],
                        scalar1=scls[kc][:], scalar2=shs[kc][:],
                        op0=mybir.AluOpType.mult, op1=mybir.AluOpType.add)

            # HAM warm-up: keep the PE busy through the phase-1 tail so
            # phase 2 starts at 2.4 GHz instead of the cold 1.2 GHz.
            warm_ps = ps.tile([P, 2, 512], F32, tag="pair", name="warm")
            for wi in range(8):
                nc.tensor.matmul(warm_ps[:, 0, :], ident_sb[:], x_full[:, 2, :512],
                                 start=(wi == 0), stop=(wi == 7))

            # weights (first needed by phase 2)
            w_sb = {}
            for name, ap in (("wq", wq8), ("wk", wk8), ("wvo", wvo8)):
                t = consts.tile([P, KC, C], DT8, tag=f"w_{name}")
                nc.gpsimd.dma_start(t[:], ap.rearrange("(kc p) o -> p kc o", p=P))
                w_sb[name] = t

            # ---- phase 2: projections (all DR fp8) ------------------------
            k_sb = big.tile([P, KC, N], DT8, tag="k")
            q_sb = big.tile([P, KC, NH], DT8, tag="q")
            v_sb = big.tile([P, KT, C], DT8, tag="v")

            def stage_q(ts, copy_eng):
                """q projection for token chunk ts (512 tokens)."""
                for oc in range(KC):
                    pp = ps.tile([P, 2, 512], F32, tag="pair")
                    for j in range(2):
                        nc.tensor.matmul(
                            pp[:, 0, :], w_sb["wq"][:, 2 * j:2 * j + 2,
                                                    oc * P:(oc + 1) * P],
                            hn[:, 2 * j:2 * j + 2, ts * 512:(ts + 1) * 512],
                            start=(j == 0), stop=(j == 1), perf_mode=DR)
                    dst = q_sb[:, oc, ts * 512:(ts + 1) * 512]
                    if copy_eng == "act":
                        nc.scalar.activation(
                            dst, pp[:, 0, :],
                            mybir.ActivationFunctionType.Identity,
                            bias=b_sb["bq"][:, oc:oc + 1], scale=1.0)
                    else:
                        nc.vector.tensor_scalar(
                            out=dst, in0=pp[:, 0, :],
                            scalar1=b_sb["bq"][:, oc:oc + 1], scalar2=None,
                            op0=mybir.AluOpType.add)

            def stage_k(ts):
                for oc in range(KC):
                    pp = ps.tile([P, 2, 512], F32, tag="pair")
                    for j in range(2):
                        nc.tensor.matmul(
                            pp[:, 0, :], w_sb["wk"][:, 2 * j:2 * j + 2,
                                                    oc * P:(oc + 1) * P],
                            hn[:, 2 * j:2 * j + 2, ts * 512:(ts + 1) * 512],
                            start=(j == 0), stop=(j == 1), perf_mode=DR)
                    dst = k_sb[:, oc, ts * 512:(ts + 1) * 512]
                    if oc % 2 == 0:
                        nc.vector.tensor_copy(dst, pp[:, 0, :])
                    else:
                        nc.scalar.copy(dst, pp[:, 0, :])

            def stage_v(jc):
                """v' projection for token chunk jc (128 tokens):
                out [tok128, C] = hn-chunk^T @ wvo."""
                pp = ps.tile([P, 2, 512], F32, tag="pair")
                for j in range(2):
                    nc.tensor.matmul(
                        pp[:, 0, :], hn[:, 2 * j:2 * j + 2, jc * P:(jc + 1) * P],
                        w_sb["wvo"][:, 2 * j:2 * j + 2, :],
                        start=(j == 0), stop=(j == 1), perf_mode=DR)
                if jc % 2 == 0:
                    nc.vector.tensor_copy(v_sb[:, jc, :], pp[:, 0, :])
                else:
                    nc.scalar.copy(v_sb[:, jc, :], pp[:, 0, :])

            # q chunk 0 first (unlocks attention), then all of k.
            stage_q(0, "act")
            for ts in range(8):
                stage_k(ts)

            # ---- phase 3: attention ---------------------------------------
            # Per query chunk qc: B1 = S^T pairs + exp into the e2 cache;
            # B2 = denom burst + recip + O' accumulation + epilogue.
            # v' projection and q chunks 1..3 are interleaved into B1(qc0).
            def b1(qc, extra):
                e2 = e2p.tile([P, KT, 512], DT8, tag="e2", name=f"e2_{qc}")
                for u in range(KT // 2):
                    s2 = ps.tile([P, 2, 512], F32, tag="pair")
                    for h in range(2):
                        kt = 2 * u + h
                        for j in range(2):
                            nc.tensor.matmul(
                                s2[:, h, :],
                                k_sb[:, 2 * j:2 * j + 2, kt * P:(kt + 1) * P],
                                q_sb[:, 2 * j:2 * j + 2,
                                     qc * 512:(qc + 1) * 512],
                                start=(j == 0), stop=(j == 1), perf_mode=DR)
                    nc.scalar.activation(
                        e2[:, 2 * u:2 * u + 2, :], s2[:],
                        mybir.ActivationFunctionType.Exp,
                        bias=ebias[:], scale=SCALE / (WSC * WSC))
                    if extra is not None:
                        extra(u)
                return e2

            def b2(qc, e2):
                # denominator burst: ones-stationary matmul over cached e2
                d_ps = ps.tile([P, 2, 512], F32, tag="pair", name=f"d{qc}")
                for u in range(KT // 2):
                    nc.tensor.matmul(
                        d_ps[:, 0, :], ones8_sb[:], e2[:, 2 * u:2 * u + 2, :],
                        start=(u == 0), stop=(u == KT // 2 - 1), perf_mode=DR)
                recip = rp.tile([P, 512], F32, tag="recip")
                nc.vector.reciprocal(recip[:], d_ps[:, 0, :])

                o01 = ps.tile([P, 2, 512], F32, tag="pair", name=f"o01_{qc}")
                o23 = ps.tile([P, 2, 512], F32, tag="pair", name=f"o23_{qc}")
                oh = [o01[:, 0, :], o01[:, 1, :], o23[:, 0, :], o23[:, 1, :]]
                for u in range(KT // 2):
                    for cc in range(KC):
                        nc.tensor.matmul(
                            oh[cc], v_sb[:, 2 * u:2 * u + 2, cc * P:(cc + 1) * P],
                            e2[:, 2 * u:2 * u + 2, :],
                            start=(u == 0), stop=(u == KT // 2 - 1),
                            perf_mode=DR)
                for cc in range(KC):
                    r32 = rp.tile([P, 512], F32, tag="r32")
                    nc.vector.tensor_tensor(
                        r32[:], oh[cc], recip[:], mybir.AluOpType.mult)
                    nc.vector.tensor_scalar(
                        out=r32[:], in0=r32[:], scalar1=1.0 / WSC,
                        scalar2=b_sb["bc"][:, cc:cc + 1],
                        op0=mybir.AluOpType.mult, op1=mybir.AluOpType.add)
                    y16 = rp.tile([P, 512], DT, tag="y16")
                    nc.vector.tensor_tensor(
                        y16[:], r32[:], x_full[:, cc, qc * 512:(qc + 1) * 512],
                        mybir.AluOpType.add)
                    nc.sync.dma_start(
                        y_t[cc][:, qc * 512:(qc + 1) * 512], y16[:])

            # qc0's B1 carries the v' projection + q chunks 1..3
            def extra_qc0(u):
                stage_v(2 * u)
                stage_v(2 * u + 1)
                if u < 12 and u % 4 == 3:
                    stage_q(1 + u // 4, "dve")

            e2 = b1(0, extra_qc0)
            for qc in range(QC):
                b2(qc, e2)
                if qc + 1 < QC:
                    e2 = b1(qc + 1, None)

    return nc


def _prep_in_maps(inputs):
    import ml_dtypes
    F8 = ml_dtypes.float8_e4m3

    x = np.asarray(inputs["x"], np.float32).reshape(4, C, N)
    wq = np.asarray(inputs["wq"], np.float32)
    wk = np.asarray(inputs["wk"], np.float32)
    wv = np.asarray(inputs["wv"], np.float32)
    wo = np.asarray(inputs["wo"], np.float32)
    wvo = wo @ wv
    bvec = np.stack([
        np.asarray(inputs["bq"], np.float32) * WSC,
        wo @ np.asarray(inputs["bv"], np.float32)
        + np.asarray(inputs["bo"], np.float32),
        np.asarray(inputs["gamma"], np.float32),
        np.asarray(inputs["beta"], np.float32),
    ]).astype(np.float32)
    shared = {
        "wq8": np.ascontiguousarray(wq.T * WSC).astype(F8),
        "wk8": np.ascontiguousarray(wk.T * WSC).astype(F8),
        "wvo8": np.ascontiguousarray(wvo.T * WSC).astype(F8),
        "bvec": bvec,
        "gavg": (np.kron(np.eye(P // GS, dtype=np.float32),
                         np.ones((GS, GS), np.float32)) / (GS * N)),
        "ident": np.eye(P, dtype=np.float16),
    }
    in_maps = []
    for core in range(N_CORES):
        b, half = divmod(core, 2)
        xb = x[b]
        if half == 1:
            xrot = np.ascontiguousarray(
                np.concatenate([xb[:, NH:], xb[:, :NH]], axis=1))
        else:
            xrot = np.ascontiguousarray(xb)
        in_maps.append({"xr": xrot.astype(np.float16), **shared})
    return in_maps


def kernel_run(inputs, trace=False, trace_cores=None):
    """Run on all 8 cores; returns (full_output, BassKernelResults)."""
    from concourse.bass_utils import run_bass_kernel_spmd

    if "nc" not in _CACHE:
        _CACHE["nc"] = _build()
    nc = _CACHE["nc"]
    in_maps = _prep_in_maps(inputs)
    res = run_bass_kernel_spmd(
        nc, in_maps, core_ids=list(range(N_CORES)), trace=trace,
        trace_cores=trace_cores)
    out = np.empty((4, C, N), np.float32)
    for core in range(N_CORES):
        b, half = divmod(core, 2)
        out[b][:, half * NH:(half + 1) * NH] = res.results[core]["y"]
    return out.reshape(4, C, 64, 64), res


def kernel(**inputs):
    out, _ = kernel_run(inputs, trace=False)
    return out


# revision 20
# speedup vs baseline: 1.8316x; 1.2304x over previous
"""AttnBlock (GroupNorm + single-head 1x1-conv attention + residual) on 8
Trainium2 NeuronCores.

Sharding: data-parallel over batch (4) x sequence-parallel over query tokens
(2 halves of 4096). Each core receives its batch element with the spatial
columns rotated so that its 2048 query tokens are always columns 0:2047 —
attention is invariant to key order, so one shared NEFF serves all cores.

All heavy matmuls run in fp8e4 with MatmulPerfMode.DoubleRow (2 contraction
chunks of 128 per instruction = 2x PE throughput). Attention uses the
transposed dataflow S^T[key, query]: exp is applied in [k, q] layout so no
PE transposes are needed, the softmax denominator comes from a ones-
stationary matmul, and the output projection is pre-fused into the v
projection on the host (wvo = wo @ wv), eliminating phase 4 entirely.

Scaling: weights are scaled x32 on the host to keep fp8 values in the
normal range; the 1/1024 (q,k) descale is folded into the exp scale and
the 1/32 (v') descale into the output epilogue.
"""

import numpy as np

P = 128
C = 512
KC = C // P          # 4 channel chunks of 128
N = 4096             # tokens (64*64)
NH = N // 2          # query tokens per core
G = 32               # groupnorm groups
GS = C // G          # 16 channels per group
EPS = 1e-6
N_CORES = 8
QW = 512             # query chunk width (PSUM bank = 512 fp32 caps matmul out)
QC = NH // QW        # 4 query chunks
KT = N // P          # 32 key chunks of 128
SCALE = float(C) ** -0.5
WSC = 32.0           # host-side weight scale
LN8 = 2.0794415416798357

_CACHE = {}


def _apply_walrus_workarounds():
    """The walrus build in this container rejects any instruction carrying
    more than one semaphore wait ("Too many sync wait commands"). Split extra
    waits onto same-engine single-wait NOPs committed just before, and split
    the final TileContext drain the same way."""
    import concourse.tile as tile
    from concourse import mybir

    if getattr(tile.TileContext, "_walrus_wait_split", False):
        return

    _orig_commit = tile.TileContext._commit_instruction

    def _split_waits_commit(self, inst, lazy_reg_writes=True):
        si = inst.sync_info
        if si is not None and si.on_wait and len(si.on_wait) > 1 \
                and inst.engine != mybir.EngineType.Unassigned:
            waits = list(si.on_wait)
            si.on_wait = waits[-1:]
            for w in waits[:-1]:
                nop = mybir.InstNoOp(
                    name=self.nc.get_next_instruction_name(),
                    engine=inst.engine,
                    sync_info=mybir.SyncInfo(on_wait=[w], on_update=[]),
                    bass_nofuse=True,
                )
                _orig_commit(self, nop, lazy_reg_writes=False)
        return _orig_commit(self, inst, lazy_reg_writes=lazy_reg_writes)

    def _split_drain_and_barrier(self, tick_clock, wait_clock):
        nc = self.nc
        drain_inst = nc.sync.drain()
        wait_clock.add_sem_waits(
            drain_inst.ins, tile.ScopedClock({None: tick_clock.global_clock})
        )
        si = drain_inst.ins.sync_info
        waits = list(si.on_wait) if si is not None else []
        if len(waits) > 1:
            si.on_wait = waits[:1]
            for w in waits[1:]:
                d2 = nc.sync.drain()
                d2.ins.sync_info = mybir.SyncInfo(on_wait=[w], on_update=[])

        import os
        nc.all_engine_barrier()
        assert self.sems is not None
        popped = nc._tile_sem_poison_stack.pop()
        assert popped is self._sem_poison
        if os.environ.get("KERNEL_SKIP_SEM_RESET") != "1":
            nc.clear_and_free_semaphores(list(self.sems.allocated().values()))
            nc.all_engine_barrier()

    tile.TileContext._commit_instruction = _split_waits_commit
    tile.TileContext._drain_and_barrier = _split_drain_and_barrier
    tile.TileContext._walrus_wait_split = True


def _build():
    """Trace the Bass/Tile program once; returns the Bass module."""
    import concourse.bass as bass
    import concourse.tile as tile
    from concourse import mybir

    _apply_walrus_workarounds()

    DR = mybir.MatmulPerfMode.DoubleRow
    DT8 = mybir.dt.float8e4
    DT = mybir.dt.float16
    F32 = mybir.dt.float32
    AT = mybir.AluOpType

    nc = bass.Bass("TRN2", target_bir_lowering=False, debug=False, num_devices=1)

    xr = nc.dram_tensor("xr", [C, N], DT, kind="ExternalInput").ap()
    wq8 = nc.dram_tensor("wq8", [C, C], DT8, kind="ExternalInput").ap()
    wk8 = nc.dram_tensor("wk8", [C, C], DT8, kind="ExternalInput").ap()
    wvo8 = nc.dram_tensor("wvo8", [C, C], DT8, kind="ExternalInput").ap()
    # packed per-channel vectors: [32*bq, wo@bv+bo, gamma, beta]
    bvec = nc.dram_tensor("bvec", [4, C], F32, kind="ExternalInput").ap()
    gavg = nc.dram_tensor("gavg", [P, P], F32, kind="ExternalInput").ap()
    ident = nc.dram_tensor("ident", [P, P], DT, kind="ExternalInput").ap()
    y = nc.dram_tensor("y", [C, NH], DT, kind="ExternalOutput").ap()

    xr_t = xr.rearrange("(kc p) n -> kc p n", p=P)     # [4, 128, 4096]
    y_t = y.rearrange("(oc p) n -> oc p n", p=P)       # [4, 128, 2048]

    with tile.TileContext(nc) as tc:
        import contextlib
        ctx = contextlib.ExitStack()
        with ctx:
            consts = ctx.enter_context(tc.tile_pool(name="consts", bufs=1))
            big = ctx.enter_context(tc.tile_pool(name="big", bufs=1))
            scp = ctx.enter_context(tc.tile_pool(name="scp", bufs=2))
            small = ctx.enter_context(tc.tile_pool(name="small", bufs=4))
            rp = ctx.enter_context(tc.tile_pool(name="rp", bufs=2))
            ps = ctx.enter_context(tc.tile_pool(name="ps", bufs=4, space="PSUM"))

            # ---- phase 1: GroupNorm -> hn (fp8) --------------------------
            # Per-chunk stats both via DVE tensor_tensor_reduce (sum uses
            # (x+x)*0.5 to stay on the 2-byte fast path); normalize mostly on
            # ACT, chunk 3 split ACT/DVE to shorten the critical path.
            hn = big.tile([P, KC, N], DT8, tag="hn")
            x_full = big.tile([P, KC, N], DT, tag="xf")
            bv_sb = None
            for kc in range(KC):
                x_c = x_full[:, kc, :]
                nc.sync.dma_start(x_c[:], xr_t[kc])
                if bv_sb is None:
                    bv_sb = consts.tile([P, 4, KC], F32, tag="bvec")
                    nc.gpsimd.dma_start(
                        bv_sb[:], bvec.rearrange("v (kc p) -> p v kc", p=P))
                    b_sb = {n: bv_sb[:, vi, :] for vi, n in
                            enumerate(("bq", "bc", "gam", "bet"))}
                    gavg_sb = consts.tile([P, P], F32, tag="gavg")
                    nc.gpsimd.dma_start(gavg_sb[:], gavg)
                    ident_sb = consts.tile([P, P], DT, tag="ident")
                    nc.gpsimd.dma_start(ident_sb[:], ident)
                    ones8_sb = consts.tile([P, 2, P], DT8, tag="ones8")
                    nc.vector.memset(ones8_sb[:], 1.0)
                    eps_sb = consts.tile([P, 1], F32, tag="eps")
                    nc.vector.memset(eps_sb[:], EPS)
                    ebias = consts.tile([P, 1], F32, tag="ebias")
                    nc.vector.memset(ebias[:], -LN8)

            stats = []
            for kc in range(KC):
                x_c = x_full[:, kc, :]
                mv2 = small.tile([P, 2], F32, tag="mv2", name=f"mv2_{kc}")
                # hierarchical sum: fp16 halves add (DVE 2x path) + reduce
                sc = scp.tile([P, N // 2], DT, tag="sc")
                nc.vector.tensor_tensor(
                    sc[:], x_c[:, :N // 2], x_c[:, N // 2:], AT.add)
                nc.vector.tensor_reduce(
                    mv2[:, 0:1], sc[:], mybir.AxisListType.X, AT.add)
                # hn[:, kc] is throwaway scratch here, overwritten below
                nc.scalar.activation(
                    hn[:, kc, 0:N], x_c[:], mybir.ActivationFunctionType.Square,
                    accum_out=mv2[:, 1:2])
                g_ps = ps.tile([P, 2], F32, tag="pair", name=f"gn{kc}")
                nc.tensor.matmul(g_ps[:], gavg_sb[:], mv2[:], start=True, stop=True)

                g_sb = small.tile([P, 2], F32, tag="gsb")
                nc.vector.tensor_copy(g_sb[:], g_ps[:])
                var_t = small.tile([P, 1], F32, tag="var")
                nc.gpsimd.tensor_tensor(
                    var_t[:], g_sb[:, 0:1], g_sb[:, 0:1], AT.mult)
                nc.gpsimd.tensor_tensor(
                    var_t[:], g_sb[:, 1:2], var_t[:], AT.subtract)
                sq = small.tile([P, 1], F32, tag="sq")
                nc.scalar.activation(
                    sq[:], var_t[:], mybir.ActivationFunctionType.Sqrt,
                    bias=eps_sb[:], scale=1.0)
                rstd = small.tile([P, 1], F32, tag="rstd")
                nc.vector.reciprocal(rstd[:], sq[:])

                scl = small.tile([P, 1], F32, tag="scl", name=f"scl{kc}")
                nc.gpsimd.tensor_tensor(
                    scl[:], rstd[:], b_sb["gam"][:, kc:kc + 1], AT.mult)
                sh = small.tile([P, 1], F32, tag="sh", name=f"sh{kc}")
                nc.gpsimd.tensor_tensor(
                    sh[:], g_sb[:, 0:1], scl[:], AT.mult)
                nc.gpsimd.tensor_tensor(
                    sh[:], b_sb["bet"][:, kc:kc + 1], sh[:], AT.subtract)
                stats.append((scl, sh))

                # normalize this chunk right away
                if kc < 2:
                    nc.gpsimd.tensor_scalar(
                        out=hn[:, kc, :], in0=x_c[:],
                        scalar1=scl[:], scalar2=sh[:],
                        op0=AT.mult, op1=AT.add)
                elif kc == 2:
                    nc.scalar.activation(
                        hn[:, kc, :], x_c[:],
                        mybir.ActivationFunctionType.Identity,
                        bias=sh[:], scale=scl[:])
                else:
                    nc.scalar.activation(
                        hn[:, kc, :N // 2], x_c[:, :N // 2],
                        mybir.ActivationFunctionType.Identity,
                        bias=sh[:], scale=scl[:])
                    nc.vector.tensor_scalar(
                        out=hn[:, kc, N // 2:], in0=x_c[:, N // 2:],
                        scalar1=scl[:], scalar2=sh[:],
                        op0=AT.mult, op1=AT.add)

            # HAM warm-up: keep the PE busy through the phase-1 tail.
            warm_ps = ps.tile([P, 512], F32, tag="pair", name="warm")
            for wi in range(8):
                nc.tensor.matmul(warm_ps[:], ident_sb[:], x_full[:, 2, :512],
                                 start=(wi == 0), stop=(wi == 7))

            w_sb = {}
            for name, ap in (("wq", wq8), ("wk", wk8), ("wvo", wvo8)):
                t = consts.tile([P, KC, C], DT8, tag=f"w_{name}")
                nc.gpsimd.dma_start(t[:], ap.rearrange("(kc p) o -> p kc o", p=P))
                w_sb[name] = t

            # ---- phase 2: projections (all DR fp8, 1024-wide) -------------
            k_sb = big.tile([P, KC, N], DT8, tag="k")
            q_sb = big.tile([P, KC, NH], DT8, tag="q")
            v_sb = big.tile([P, KT, C], DT8, tag="v")

            def stage_kq(w, dst_sb, tsp, bias, eng):
                """projection of token chunk tsp (1024 tokens) -> dst."""
                for ocp in range(2):
                    pp = ps.tile([P, 2, QW], F32, tag="pair")
                    for h in range(2):
                        oc = 2 * ocp + h
                        for j in range(2):
                            nc.tensor.matmul(
                                pp[:, h, :],
                                w[:, 2 * j:2 * j + 2, oc * P:(oc + 1) * P],
                                hn[:, 2 * j:2 * j + 2,
                                   tsp * QW:(tsp + 1) * QW],
                                start=(j == 0), stop=(j == 1), perf_mode=DR)
                    for h in range(2):
                        oc = 2 * ocp + h
                        dst = dst_sb[:, oc, tsp * QW:(tsp + 1) * QW]
                        e = eng(oc)
                        if bias is None:
                            if e == "act":
                                nc.scalar.copy(dst, pp[:, h, :])
                            else:
                                nc.vector.tensor_copy(dst, pp[:, h, :])
                        else:
                            if e == "act":
                                nc.scalar.activation(
                                    dst, pp[:, h, :],
                                    mybir.ActivationFunctionType.Identity,
                                    bias=bias[:, oc:oc + 1], scale=1.0)
                            else:
                                nc.vector.tensor_scalar(
                                    out=dst, in0=pp[:, h, :],
                                    scalar1=bias[:, oc:oc + 1], scalar2=None,
                                    op0=AT.add)

            def stage_v_pair(u):
                """v' projection for token chunks 2u, 2u+1 (128 tokens each)."""
                pp = ps.tile([P, 2, 512], F32, tag="pair")
                for h in range(2):
                    jc = 2 * u + h
                    for j in range(2):
                        nc.tensor.matmul(
                            pp[:, h, :],
                            hn[:, 2 * j:2 * j + 2, jc * P:(jc + 1) * P],
                            w_sb["wvo"][:, 2 * j:2 * j + 2, :],
                            start=(j == 0), stop=(j == 1), perf_mode=DR)
                for h in range(2):
                    jc = 2 * u + h
                    if jc % 2 == 0:
                        nc.vector.tensor_copy(v_sb[:, jc, :], pp[:, h, :])
                    else:
                        nc.scalar.copy(v_sb[:, jc, :], pp[:, h, :])

            # q chunk 0 first (unlocks attention), then all of k.
            stage_kq(w_sb["wq"], q_sb, 0, b_sb["bq"], lambda oc: "act")
            for tsp in range(N // QW):
                stage_kq(w_sb["wk"], k_sb, tsp, None,
                         lambda oc: "dve" if oc % 2 == 0 else "act")

            # ---- phase 3: attention ---------------------------------------
            def b1(qc, extra):
                e2 = e2p.tile([P, KT, QW], DT8, tag="e2", name=f"e2_{qc}")
                for u in range(KT // 2):
                    s2 = ps.tile([P, 2, QW], F32, tag="pair")
                    for h in range(2):
                        kt = 2 * u + h
                        for j in range(2):
                            nc.tensor.matmul(
                                s2[:, h, :],
                                k_sb[:, 2 * j:2 * j + 2, kt * P:(kt + 1) * P],
                                q_sb[:, 2 * j:2 * j + 2,
                                     qc * QW:(qc + 1) * QW],
                                start=(j == 0), stop=(j == 1), perf_mode=DR)
                    nc.scalar.activation(
                        e2[:, 2 * u:2 * u + 2, :], s2[:],
                        mybir.ActivationFunctionType.Exp,
                        bias=ebias[:], scale=SCALE / (WSC * WSC))
                    if extra is not None:
                        extra(u)
                return e2

            def b2(qc, e2):
                d_ps = ps.tile([P, 2, QW], F32, tag="pair", name=f"d{qc}")
                for u in range(KT // 2):
                    nc.tensor.matmul(
                        d_ps[:, 0, :], ones8_sb[:], e2[:, 2 * u:2 * u + 2, :],
                        start=(u == 0), stop=(u == KT // 2 - 1), perf_mode=DR)
                recip = rp.tile([P, QW], F32, tag="recip")
                nc.vector.reciprocal(recip[:], d_ps[:, 0, :])

                for ccp in range(2):
                    o2 = ps.tile([P, 2, QW], F32, tag="pair",
                                 name=f"o{ccp}_{qc}")
                    for u in range(KT // 2):
                        for h in range(2):
                            cc = 2 * ccp + h
                            nc.tensor.matmul(
                                o2[:, h, :],
                                v_sb[:, 2 * u:2 * u + 2, cc * P:(cc + 1) * P],
                                e2[:, 2 * u:2 * u + 2, :],
                                start=(u == 0), stop=(u == KT // 2 - 1),
                                perf_mode=DR)
                    for h in range(2):
                        cc = 2 * ccp + h
                        r32 = rp.tile([P, QW], F32, tag="r32")
                        nc.vector.tensor_tensor(
                            r32[:], o2[:, h, :], recip[:], AT.mult)
                        nc.vector.tensor_scalar(
                            out=r32[:], in0=r32[:], scalar1=1.0 / WSC,
                            scalar2=b_sb["bc"][:, cc:cc + 1],
                            op0=AT.mult, op1=AT.add)
                        y16 = rp.tile([P, QW], DT, tag="y16")
                        nc.vector.tensor_tensor(
                            y16[:], r32[:],
                            x_full[:, cc, qc * QW:(qc + 1) * QW], AT.add)
                        nc.sync.dma_start(
                            y_t[cc][:, qc * QW:(qc + 1) * QW], y16[:])

            e2p = big  # single-buffered: strict B2(qc) -> B1(qc+1) PE order
            e2 = b1(0, None)
            # v' projection + q chunks 1..3 fill the PE while b1(0) drains
            for u in range(KT // 2):
                stage_v_pair(u)
                if u % 4 == 3 and u // 4 + 1 < QC:
                    stage_kq(w_sb["wq"], q_sb, u // 4 + 1, b_sb["bq"],
                             lambda oc: "dve" if oc % 2 == 0 else "act")
            for qc in range(QC):
                b2(qc, e2)
                if qc + 1 < QC:
                    e2 = b1(qc + 1, None)

    return nc


def _prep_in_maps(inputs):
    import ml_dtypes
    F8 = ml_dtypes.float8_e4m3

    x = np.asarray(inputs["x"], np.float32).reshape(4, C, N)
    wq = np.asarray(inputs["wq"], np.float32)
    wk = np.asarray(inputs["wk"], np.float32)
    wv = np.asarray(inputs["wv"], np.float32)
    wo = np.asarray(inputs["wo"], np.float32)
    wvo = wo @ wv
    bvec = np.stack([
        np.asarray(inputs["bq"], np.float32) * WSC,
        wo @ np.asarray(inputs["bv"], np.float32)
        + np.asarray(inputs["bo"], np.float32),
        np.asarray(inputs["gamma"], np.float32),
        np.asarray(inputs["beta"], np.float32),
    ]).astype(np.float32)
    shared = {
        "wq8": np.ascontiguousarray(wq.T * WSC).astype(F8),
        "wk8": np.ascontiguousarray(wk.T * WSC).astype(F8),
        "wvo8": np.ascontiguousarray(wvo.T * WSC).astype(F8),
        "bvec": bvec,
        "gavg": (np.kron(np.eye(P // GS, dtype=np.float32),
                         np.ones((GS, GS), np.float32)) / (GS * N)),
        "ident": np.eye(P, dtype=np.float16),
    }
    in_maps = []
    for core in range(N_CORES):
        b, half = divmod(core, 2)
        xb = x[b]
        if half == 1:
            xrot = np.ascontiguousarray(
                np.concatenate([xb[:, NH:], xb[:, :NH]], axis=1))
        else:
            xrot = np.ascontiguousarray(xb)
        in_maps.append({"xr": xrot.astype(np.float16), **shared})
    return in_maps


def kernel_run(inputs, trace=False, trace_cores=None):
    """Run on all 8 cores; returns (full_output, BassKernelResults)."""
    from concourse.bass_utils import run_bass_kernel_spmd

    if "nc" not in _CACHE:
        _CACHE["nc"] = _build()
    nc = _CACHE["nc"]
    in_maps = _prep_in_maps(inputs)
    res = run_bass_kernel_spmd(
        nc, in_maps, core_ids=list(range(N_CORES)), trace=trace,
        trace_cores=trace_cores)
    out = np.empty((4, C, N), np.float32)
    for core in range(N_CORES):
        b, half = divmod(core, 2)
        out[b][:, half * NH:(half + 1) * NH] = res.results[core]["y"]
    return out.reshape(4, C, 64, 64), res


def kernel(**inputs):
    out, _ = kernel_run(inputs, trace=False)
    return out
